# revision 1
# baseline (speedup 1.0000x reference)
"""Forward-fill imputation + missing indicators (MissingValueHandlerLayer).

Input : x (128, 2048, 64) f32, missing entries are exactly 0.0
Output: (128, 2048, 128) f32 = concat([forward_filled(x), (x==0).f32], axis=-1)

Math: with ind[t] = (x[t]==0), the forward fill is the affine recurrence
    imp[t] = ind[t]*imp[t-1] + x[t]     (imp[-1] = 0)
which is exactly one VectorE tensor_tensor_scan (op0=mult, op1=add) along
the free dim.  Per core: 16 batches, processed as 8 batch-pairs so that
128 partitions = 2 batches x 64 feature-series; PE transposes move between
the natural (t-major) layout and the series layout.
"""

import os

import numpy as np

B, T, F = 128, 2048, 64
N_CORES = 8
B_LOC = B // N_CORES  # 16 batches per core
NPAIRS = B_LOC // 2   # 8
NT = T // 128         # 16 t-blocks of 128
NCH = 4               # chunks of 4 t-blocks (512 cols) for PSUM staging

_module = None


def _build_module(n_batches=B_LOC, repeats=1, mode="full"):
    import concourse.bacc as bacc
    import concourse.tile as tile
    from concourse import mybir
    from concourse.masks import make_identity

    do_in = mode in ("full", "in", "dma")
    do_compute = mode in ("full", "compute", "pe", "pescan", "outnogp")
    do_eq_copy = mode in ("full", "compute", "pescan", "outnogp")
    do_scan = mode in ("full", "compute", "pescan", "outnogp")
    do_out_half = mode in ("full", "compute", "outnogp")
    do_gp = mode in ("full", "compute", "gp")
    do_out = mode in ("full", "out", "dma")

    npairs = n_batches // 2
    FP = mybir.dt.float32
    nc = bacc.Bacc(
        "TRN2", target_bir_lowering=False, debug=False, num_devices=N_CORES
    )
    x = nc.dram_tensor("x", (n_batches, T, F), FP, kind="ExternalInput").ap()
    out = nc.dram_tensor("out", (n_batches, T, 2 * F), FP, kind="ExternalOutput").ap()

    MUL = mybir.AluOpType.mult
    ADD = mybir.AluOpType.add
    EQ = mybir.AluOpType.is_equal

    with tile.TileContext(nc) as tc:
        with (
            tc.tile_pool(name="consts", bufs=1) as consts,
            tc.tile_pool(name="sload", bufs=5) as sload,
            tc.tile_pool(name="scanbuf", bufs=3) as scanbuf,
            tc.tile_pool(name="pin", bufs=4, space="PSUM") as pin,
            tc.tile_pool(name="pout", bufs=4, space="PSUM") as pout,
            tc.tile_pool(name="obuf", bufs=4) as obuf,
        ):
            ident = consts.tile([128, 128], FP)
            make_identity(nc, ident)

            persist_O = []
            if do_out and not do_compute:
                for i in range(2):
                    Op = consts.tile(
                        [128, 2, NT, 2 * F], FP, tag=f"Opersist{i}", name=f"Op{i}"
                    )
                    nc.vector.memset(Op, 0.25)
                    persist_O.append(Op)
            if not do_out:
                # token write so the ExternalOutput has a producer
                nc.sync.dma_start(
                    out=out[0, 0:128, :], in_=ident
                )

            for p in range(npairs * repeats):
                p = p % npairs
                S = None
                if do_in or do_compute or do_gp:
                    # S[q, (u, b2, f)] = x[2p+b2, 16q+u, f]: partition
                    # q = t div 16.  The b2-interleave keeps each u-slice of
                    # the free dim equal to (b2, f) = 128 contiguous, which is
                    # what the PE transpose needs (its weight AP allows only
                    # one free dim, and transpose outputs must start at PSUM
                    # partition 0 — so a batch-major layout is not possible).
                    # (A batch-major staging tile + on-chip permute copy was
                    # tried: the input DMA sped up but the scattered-src copy
                    # cost more than it saved — full went 82 -> 119 us.)
                    S = sload.tile([128, T], FP, tag="S", name=f"S{p}")
                    Sv = S.rearrange("q (u b2 f) -> q u b2 f", u=16, b2=2)
                if do_in:
                    # one 1MB load per pair (4D src AP interleaving both
                    # batches).  (Splitting loads across HWDGE-SP + SWDGE-Pool
                    # was tried and measured neutral-to-worse: 86-89 us vs
                    # 82-85 us; single SP ring kept.)
                    nc.sync.dma_start(
                        out=Sv,
                        in_=x[2 * p:2 * p + 2].rearrange(
                            "b2 (q u) f -> q u b2 f", u=16
                        ),
                    )
                elif do_compute or do_gp:
                    # mark the tile written so Tile allocates it (timing-only
                    # mode; compute then reads whatever SBUF holds)
                    nc.vector.memset(S[:, 0:8], 0.0)

                O = None
                if do_compute:
                    # Series layout: partition = b2*64+f, free = t
                    if do_eq_copy:
                        xT = scanbuf.tile([128, T], FP, tag="xT", name=f"xT{p}")
                        indT = scanbuf.tile([128, T], FP, tag="indT", name=f"indT{p}")
                        xTu = xT.rearrange("p (k u) -> p u k", u=16)
                        indTu = indT.rearrange("p (k u) -> p u k", u=16)
                    if do_scan:
                        impT = scanbuf.tile([128, T], FP, tag="impT", name=f"impT{p}")

                    for c in range(NCH):
                        P4 = pin.tile([128, 512], FP, tag="pin", name=f"P4_{p}_{c}")
                        for j in range(4):
                            u = 4 * c + j
                            # S free slice u is (b2, f), 128 contiguous ->
                            # P4[:, j] = [part (b2 f), free q], t = 16q+u
                            nc.tensor.transpose(
                                P4[:, j * 128:(j + 1) * 128],
                                S[:, u * 128:(u + 1) * 128],
                                ident,
                            )
                        if do_eq_copy:
                            # P4 free = (j, q) -> strided dst t = 16q + (4c+j)
                            nc.scalar.copy(
                                out=xTu[:, 4 * c:4 * c + 4, :], in_=P4
                            )
                            nc.vector.tensor_scalar(
                                out=indTu[:, 4 * c:4 * c + 4, :],
                                in0=P4,
                                scalar1=0.0,
                                scalar2=None,
                                op0=EQ,
                            )

                    if do_scan:
                        nc.vector.tensor_tensor_scan(
                            out=impT,
                            data0=indT,
                            data1=xT,
                            initial=0.0,
                            op0=MUL,
                            op1=ADD,
                        )

                if do_out_half:
                    # O[q, (b2, u, c128)] = out[2p+b2, 16q+u, c128]: partition
                    # q = t div 16 (same as S!), so each store is one
                    # fully-contiguous 1MB DMA with 8KB-per-partition chunks.
                    O = obuf.tile([128, 2, 16, 2 * F], FP, tag="O", name=f"O{p}")
                    # indicators: partition-aligned direct op S -> O on DVE
                    # (GpSimd takes ~209us for this scattered AP; DVE's HW
                    # address generators run it at line rate)
                    nc.vector.tensor_scalar(
                        out=O[:, :, :, F:],
                        in0=Sv.transpose([0, 2, 1, 3]),  # (q, b2, u, f)
                        scalar1=0.0,
                        scalar2=None,
                        op0=EQ,
                    )
                    impTu = impT.rearrange("p (k u) -> p u k", u=16)
                    for c in range(NCH):
                        Q = pout.tile([128, 512], FP, tag="pout", name=f"Q{p}_{c}")
                        for j in range(4):
                            u = 4 * c + j
                            # strided column slice t = u (mod 16) -> out
                            # partition becomes q = t div 16
                            nc.tensor.transpose(
                                Q[:, j * 128:(j + 1) * 128],
                                impTu[:, u, :],
                                ident,
                            )
                        # Q free = (j, b2, f) -> dst (b2, j, f)
                        nc.scalar.copy(
                            out=O[:, :, 4 * c:4 * c + 4, 0:F],
                            in_=Q.rearrange("q (j b2 f) -> q b2 j f", j=4, b2=2),
                        )

                if do_out:
                    Osrc = O if O is not None else persist_O[p % 2]
                    # one 2MB store per pair (dst contiguous 2 batches) on the
                    # ACT HWDGE ring, so stores don't head-of-line-block the
                    # loads on SP's ring.  (Alternating ACT/Pool-SWDGE rings
                    # was tried: 81.8 us vs 72.9 — SWDGE stores are slower.)
                    nc.scalar.dma_start(
                        out=out[2 * p:2 * p + 2].rearrange(
                            "b2 (q u) c -> q b2 u c", u=16
                        ),
                        in_=Osrc,
                    )

    nc.compile()
    return nc


def _get_module():
    global _module
    if _module is None:
        _module = _build_module()
    return _module


def _run_spmd(in_maps, **kwargs):
    from concourse import bass_utils

    nc = _get_module()
    return bass_utils.run_bass_kernel_spmd(
        nc, in_maps, core_ids=list(range(N_CORES)), **kwargs
    )


def _make_in_maps(x):
    x = np.ascontiguousarray(x, dtype=np.float32)
    assert x.shape == (B, T, F), x.shape
    return [{"x": x[i * B_LOC:(i + 1) * B_LOC]} for i in range(N_CORES)]


def kernel(x):
    res = _run_spmd(_make_in_maps(x))
    return np.concatenate([r["out"] for r in res.results], axis=0)


# ───────────────────────── timing helpers (not used for grading) ──────────


def _make_sharded_fn(nc):
    """Build the 8-core sharded jit callable for a module (mirrors
    bass2jax.run_bass_via_pjrt's multi-core branch) so inputs can stay
    device-resident across timing iterations."""
    import jax
    from jax.experimental.shard_map import shard_map
    from jax.sharding import Mesh, PartitionSpec

    from concourse.bass2jax import (
        _bass_exec_p,
        install_neuronx_cc_hook,
        partition_id_tensor,
    )

    install_neuronx_cc_hook()
    out_aval = jax.core.ShapedArray((B_LOC, T, 2 * F), np.float32)
    pname = nc.partition_id_tensor.name if nc.partition_id_tensor else None
    in_names = ("x", "out") + ((pname,) if pname else ())

    def _body(xa, za):
        operands = [xa, za]
        if pname is not None:
            operands.append(partition_id_tensor())
        outs = _bass_exec_p.bind(
            *operands,
            out_avals=(out_aval,),
            in_names=in_names,
            out_names=("out",),
            lowering_input_output_aliases=(),
            sim_require_finite=True,
            sim_require_nnan=True,
            nc=nc,
        )
        return tuple(outs)

    devices = jax.devices()[:N_CORES]
    mesh = Mesh(np.asarray(devices), ("core",))
    P = PartitionSpec("core")
    fn = jax.jit(
        shard_map(
            _body, mesh=mesh, in_specs=(P, P), out_specs=(P,), check_rep=False
        ),
        donate_argnums=(1,),
        keep_unused=True,
    )
    return fn, mesh


def timed_run(x, r_hi=9, r_lo=1, reps=10, mode="full"):
    """Returns (out_full, per_pass_ns).

    Per-dispatch overhead through the axon relay is ~1.4 ms — more than
    10x the kernel — and the compile hook allows exactly one bass_exec
    per jit, so N-chained executions per dispatch are impossible.  Instead
    build module variants whose NEFF repeats the whole kernel body R times
    (idempotent: same output rewritten), and take the slope
    (T(r_hi) - T(r_lo)) / (r_hi - r_lo): pure on-device per-pass time,
    dispatch overhead cancelled.
    """
    import time

    import jax
    from jax.sharding import NamedSharding, PartitionSpec

    x = np.ascontiguousarray(x, dtype=np.float32)

    M = int(os.environ.get("KERNEL_TIMING_M", "24"))

    def bench(repeats):
        if repeats == 1 and mode == "full":
            nc = _get_module()
        else:
            nc = _build_module(repeats=repeats, mode=mode)
        fn, mesh = _make_sharded_fn(nc)
        sh = NamedSharding(mesh, PartitionSpec("core"))
        xd = jax.device_put(x, sh)
        o = jax.device_put(np.zeros((B, T, 2 * F), np.float32), sh)
        (o,) = fn(xd, o)  # compile + warmup
        (o,) = fn(xd, o)
        o.block_until_ready()
        times = []
        for _ in range(reps):
            t0 = time.perf_counter()
            for _ in range(M):
                (o,) = fn(xd, o)
            o.block_until_ready()
            times.append(time.perf_counter() - t0)
        times.sort()
        if os.environ.get("KERNEL_TIMING_VERBOSE"):
            q = ", ".join(f"{t * 1e3:.2f}" for t in times)
            print(f"    bench(r={repeats}): ms sorted = [{q}]")
        return times[len(times) // 4], o

    t_lo, _ = bench(r_lo)
    t_hi, o = bench(r_hi)
    per_pass_ns = (t_hi - t_lo) / (M * (r_hi - r_lo)) * 1e9
    return np.asarray(o), per_pass_ns

